# revision 43
# baseline (speedup 1.0000x reference)
"""5x5 conv2d on TRN2, data-parallel over 8 cores (4 images/core).

Default: scheme A (v3) -- 25 taps x 8 quadrant matmuls per 8-row round:
per tap, 4 images (32-channel contraction, row groups 0/32/64/96) x 2
output row blocks (col groups 0/64), bf16, PSUM-accumulated over taps,
f32 output. Measured 163 us/rep steady-state on 1 core.

Optimization findings from this session (robust min-based timing,
16400-rep hardware loops; all variants numerically validated or
timing-only as noted):
- A sits at the tensor engine's quadrant-stream concurrency limit:
  3200 N=512 MMs at ~4 concurrent streams = 170 us floor; measured 163.
- LDW cost is NOT column-bound: scheme E (32-col weights, timing-only)
  measured 179 us (not ~85), refuting the weight-bus-column model.
- 128-col-wide MMs do NOT co-stream: scheme P (dx-pair packed
  stationary [32,128], 1920 MMs) measured 425 us ~= full serialization,
  despite being correct (rel err 4.0e-3) and 40% fewer instructions.
- 64-contraction (64,64)-quadrant MMs via dy-stacked input copies
  (scheme Q/R, correct at 3.6e-3) also failed to co-stream: 326 us
  slot-interleaved (825 us image-blocked -- issue order matters 2.5x);
  dropping the xr tile family (R: dy=4 row as five 32-contr solos from
  xq j=0, half the input DMAs) improved it to 275 us, and reshaping to
  the probe's exact geometry (S, current Q-branch code: 8-row rounds,
  one image PAIR per round so all 4 quadrant streams read a single xq
  tile, top/bot row blocks on anti-diagonal quadrants) reached 249 us
  -- still 1.5x slower than A, though the scheme-A-shaped timing probe
  with 64-deep lhsT from the plain xb band tile measured 149 us. The
  remaining differences vs that probe: stacked-copy rhs tiles (vs the
  natural band tile), 15 slots per 8 rows per pair (vs 25 shared), and
  the bf16 staged output path. Root cause unresolved -- no NTFF
  tracing available under this axon setup.
- B (2 MMs/LDW, 8 live banks) 181 us, C (col grp 0 only) 223 us,
  D 273 us: PSUM bank pressure stalls dominate any LDW amortization.
- Reordering A's per-tap emission col-major (consecutive MMs always in
  different row groups, to enable LDWEIGHTS pull-ahead) measured
  171.7 us -- no better than the original interleaved order, so LDW
  scheduling is not A's binding constraint either.

Schemes P and Q are kept for reference/further work; A is the fastest
measured correct configuration.
"""

import numpy as np
import ml_dtypes

import concourse.bass as bass
import concourse.tile as tile
from concourse import bacc, mybir
from concourse.bass_utils import run_bass_kernel_spmd

N_CORES = 8
IMGS = 4
C = 32
O = 64
H = W = 128
KH = KW = 5
PAD = 2
WP = W + 2 * PAD      # 132
BANDS = 4
BAND_OUT = H // BANDS  # 32
BAND_IN = BAND_OUT + 2 * PAD  # 36
TAPS = KH * KW
RT = 4                 # rows per psum bank round

F32 = mybir.dt.float32
BF16 = mybir.dt.bfloat16

SCHEME = "A"
DT = "bf16"


def _build_nc(reps=1, SCHEME=None, DT=None):
    if SCHEME is None:
        SCHEME = globals()["SCHEME"]
    if DT is None:
        DT = globals()["DT"]
    assert DT == "bf16"
    mm_dt = BF16
    io_dt = BF16

    nc = bacc.Bacc("TRN2", target_bir_lowering=False, debug=False)
    X = nc.dram_tensor("X", [IMGS, C, H, W], io_dt, kind="ExternalInput").ap()
    K = nc.dram_tensor("K", [O, C, KH, KW], io_dt, kind="ExternalInput").ap()
    Z = nc.dram_tensor("Z", [128, BAND_IN, 2 * PAD], io_dt, kind="ExternalInput").ap()
    ZR = nc.dram_tensor("ZR", [128, PAD, WP], io_dt, kind="ExternalInput").ap()
    out_dt = BF16 if SCHEME in ("P", "Q") else F32
    out = nc.dram_tensor("out", [IMGS, O, H, W], out_dt, kind="ExternalOutput").ap()

    taps = [(dy, dx) for dy in range(KH) for dx in range(KW)]
    # [c, dy, dx, o] view of the kernel
    ksrc5 = K.rearrange("o c h w -> c h w o")

    from contextlib import ExitStack

    with tile.TileContext(nc) as tc:
        with ExitStack() as _stk:
            wpool = _stk.enter_context(tc.tile_pool(name="wpool", bufs=1))
            opool = _stk.enter_context(tc.tile_pool(name="opool", bufs=8))
            ppool = _stk.enter_context(tc.tile_pool(name="ppool", bufs=8, space="PSUM"))
            if SCHEME == "Q":
                xqpool = _stk.enter_context(tc.tile_pool(name="xqpool", bufs=6))
                xrpool = _stk.enter_context(tc.tile_pool(name="xrpool", bufs=4))
            else:
                xpool = _stk.enter_context(tc.tile_pool(name="xpool", bufs=4))
            Xall = X.rearrange("g c h w -> (g c) h w")

            def load_band_static(b, xb):
                # interior only: pad rows/cols were zeroed once at startup
                # and are never overwritten, so they stay zero across reps.
                y0 = b * BAND_OUT
                p_lo = PAD if b == 0 else 0
                p_hi = BAND_IN - 1 - PAD if b == BANDS - 1 else BAND_IN - 1
                r_lo = y0 + p_lo - PAD
                r_hi = y0 + p_hi - PAD
                nc.sync.dma_start(
                    xb[:, p_lo : p_hi + 1, PAD : PAD + W],
                    Xall[:, r_lo : r_hi + 1, :],
                )

            def load_band(b):
                y0 = b * BAND_OUT
                xb = xpool.tile([128, BAND_IN, WP], mm_dt)
                p_lo = PAD if b == 0 else 0
                p_hi = BAND_IN - 1 - PAD if b == BANDS - 1 else BAND_IN - 1
                r_lo = y0 + p_lo - PAD
                r_hi = y0 + p_hi - PAD
                nc.sync.dma_start(xb[:, :, 0:PAD], Z[:, :, 0:PAD])
                nc.sync.dma_start(xb[:, :, PAD + W : WP], Z[:, :, PAD : 2 * PAD])
                if b == 0:
                    nc.sync.dma_start(xb[:, 0:PAD, :], ZR)
                if b == BANDS - 1:
                    nc.sync.dma_start(xb[:, BAND_IN - PAD : BAND_IN, :], ZR)
                for g in range(IMGS):
                    nc.sync.dma_start(
                        xb[32 * g : 32 * g + 32, p_lo : p_hi + 1, PAD : PAD + W],
                        X[g, :, r_lo : r_hi + 1, :],
                    )
                return xb

            if SCHEME == "Q":
                # 64-deep contraction via dy-stacked input copies; 4
                # concurrent (64,64) quadrant MMs (one per image); all
                # shifts live in the data so PSUM accumulates the full
                # conv -> eviction is a plain cast copy. 13 MM slots per
                # image per 4-row round cover all 25 taps:
                #   10: (dy-pair {0,1}/{2,3}) x dx 0..4   [64-contr]
                #    2: dy=4, dx-pair {0,1}/{2,3}         [64-contr, xr]
                #    1: dy=4, dx=4                        [32-contr]
                # wq[32j+c, Dp, dx, o] = K[o,c,2Dp+j,dx]
                # replicated on both partition halves: the matmul fmap and
                # weights must start at the same SB partition index.
                wq = wpool.tile([128, 2, KH, O], mm_dt)
                # wsol[64h+c, dx, o] = K[o,c,4,dx]  (dy=4 row, 32-contr)
                wsol = wpool.tile([128, KW, O], mm_dt)
                for hh in range(2):
                    for j in range(2):
                        base = 64 * hh + 32 * j
                        for Dp in range(2):
                            for dx in range(KW):
                                nc.sync.dma_start(
                                    wq[base : base + 32, Dp, dx, :],
                                    ksrc5[:, 2 * Dp + j, dx, :],
                                )
                    for dx in range(KW):
                        nc.sync.dma_start(
                            wsol[64 * hh : 64 * hh + 32, dx, :],
                            ksrc5[:, 4, dx, :],
                        )

                # stacked band tiles:
                # xq[64p+32j+c, r, x] = Xp[img, c, y0+r+j, x]   (rows 36)
                # xr[64p+32j+c, r, x] = Xp[img, c, y0+r+4, x+j] (rows 32)
                XQR = 36
                XRR = BAND_OUT
                if True:
                    # one-time zero of all ring buffers (side cols are
                    # never written by loads, so they stay zero forever)
                    for _ in range(6):
                        tz = xqpool.tile([128, XQR, WP], mm_dt, name="xq", tag="xq")
                        nc.vector.memzero(tz[:, :, :])

                    def load_band_q(b):
                        y0 = b * BAND_OUT
                        xqs = [xqpool.tile([128, XQR, WP], mm_dt, name="xq", tag="xq") for i in range(2)]
                        for p in range(2):  # image pair
                            for j in range(2):
                                # re-zero pad rows on edge bands (side
                                # cols are never written -> stay zero
                                # from the one-time memzero)
                                lo = max(0, 2 - y0 - j)
                                hi = min(XQR - 1 - j, (H + 1) - y0 - j)
                                for g2 in range(2):
                                    base = 64 * g2 + 32 * j
                                    if lo > 0:
                                        nc.vector.memzero(
                                            xqs[p][base : base + 32, 0:lo, :]
                                        )
                                    if hi < XQR - 1:
                                        nc.vector.memzero(
                                            xqs[p][base : base + 32, hi + 1 : XQR, :]
                                        )
                                    nc.sync.dma_start(
                                        xqs[p][base : base + 32, lo : hi + 1, PAD : PAD + W],
                                        X[
                                            2 * p + g2, :,
                                            y0 + lo + j - PAD : y0 + hi + j - PAD + 1,
                                            :,
                                        ],
                                    )
                        return xqs

                    def band_Q(b, xqs):
                        y0 = b * BAND_OUT
                        obs = [
                            opool.tile([128, 16, W], out_dt, name=f"qob{g}", tag="ob")
                            for g in range(IMGS)
                        ]
                        # probe-faithful geometry: 8-row rounds, one
                        # image PAIR per round (all 4 quadrant streams read
                        # a single xq tile), top/bot row-blocks on the
                        # anti-diagonal quadrants, 2 PSUM tiles per round.
                        for t in range(BAND_OUT // (2 * RT)):  # 4 rounds
                            ybase = 2 * RT * t
                            for p in range(2):  # image pair
                                xq = xqs[p]
                                psA = ppool.tile([128, RT, W], F32, name="spsA", tag="ps")
                                psB = ppool.tile([128, RT, W], F32, name="spsB", tag="ps")
                                slots = [("q", Dp, dx) for Dp in range(2) for dx in range(KW)]
                                slots += [("s", 0, dx) for dx in range(KW)]
                                NS = len(slots)
                                for si, (kind, a, dx) in enumerate(slots):
                                    first = si == 0
                                    last = si == NS - 1
                                    for e, bot in ((0, 0), (1, 0), (0, 1), (1, 1)):
                                        rowp = 64 * e
                                        colp = 64 * (e ^ bot)
                                        osl = (psB if bot else psA)[colp : colp + O, :, :]
                                        rb = ybase + RT * bot
                                        if kind == "q":
                                            nc.tensor.matmul(
                                                osl,
                                                wq[rowp : rowp + 64, a, dx, :],
                                                xq[
                                                    rowp : rowp + 64,
                                                    rb + 2 * a : rb + 2 * a + RT,
                                                    dx : dx + W,
                                                ],
                                                start=first, stop=last,
                                                tile_position=(rowp, colp),
                                            )
                                        else:
                                            nc.tensor.matmul(
                                                osl,
                                                wsol[rowp : rowp + 32, dx, :],
                                                xq[
                                                    rowp : rowp + 32,
                                                    rb + 4 : rb + 4 + RT,
                                                    dx : dx + W,
                                                ],
                                                start=first, stop=last,
                                                tile_position=(rowp, colp),
                                            )
                                for e in range(2):
                                    g = 2 * p + e
                                    for bot in range(2):
                                        colp = 64 * (e ^ bot)
                                        src = (psB if bot else psA)[colp : colp + O, :, :]
                                        rows0 = ybase + RT * bot
                                        h = rows0 // 16
                                        j0 = rows0 % 16
                                        dst = obs[g][O * h : O * h + O, j0 : j0 + RT, :]
                                        if (g + bot) % 2 == 0:
                                            nc.vector.tensor_copy(dst, src)
                                        else:
                                            nc.scalar.activation(
                                                dst, src,
                                                mybir.ActivationFunctionType.Copy,
                                            )
                        for g in range(IMGS):
                            dst = out[g, :, y0 : y0 + BAND_OUT, :].rearrange(
                                "o (h j) w -> h o j w", h=2
                            )
                            nc.sync.dma_start(dst, obs[g])

                    def body():
                        for b in range(BANDS):
                            xqs = load_band_q(b)
                            band_Q(b, xqs)

            elif SCHEME == "P":
                # ---- weights ----
                # pair slots: wp[32g+c, e, dy, 0:64]=K[.,c,dy,2e],
                #             wp[32g+c, e, dy, 64:128]=K[.,c,dy,2e+1]
                wp = wpool.tile([128, 2, KH, 128], mm_dt)
                # solo slots: ws[32g+c, dy, o] = K[o,c,dy,4]
                ws = wpool.tile([128, KH, O], mm_dt)
                # fixup weights: wtf[32j+c, o] = K[o,c,j,1], j=0..3
                wtf = wpool.tile([128, 1, O], mm_dt)
                wtf4 = wpool.tile([32, 1, O], mm_dt)
                for g in range(IMGS):
                    for e in range(2):
                        for dy in range(KH):
                            nc.sync.dma_start(
                                wp[32 * g : 32 * g + 32, e, dy, 0:O],
                                ksrc5[:, dy, 2 * e, :],
                            )
                            nc.sync.dma_start(
                                wp[32 * g : 32 * g + 32, e, dy, O:128],
                                ksrc5[:, dy, 2 * e + 1, :],
                            )
                    for dy in range(KH):
                        nc.sync.dma_start(
                            ws[32 * g : 32 * g + 32, dy, :], ksrc5[:, dy, 4, :]
                        )
                for j in range(4):
                    nc.sync.dma_start(
                        wtf[32 * j : 32 * j + 32, 0, :], ksrc5[:, j, 1, :]
                    )
                nc.sync.dma_start(wtf4[:, 0, :], ksrc5[:, 4, 1, :])

                # ---- fixup input strips (zeroed once; interiors re-DMAd
                # each rep, edge rows stay zero) ----
                xs = [wpool.tile([128, H, 2], mm_dt, name=f"xs{g}") for g in range(IMGS)]
                xs4 = [wpool.tile([32, H, 2], mm_dt, name=f"xs4{g}") for g in range(IMGS)]
                fixsb = [wpool.tile([O, H, 1], F32, name=f"fsb{g}") for g in range(IMGS)]
                for g in range(IMGS):
                    nc.vector.memzero(xs[g][:, :, :])
                    nc.vector.memzero(xs4[g][:, :, :])

                # dedicated per-band buffers, pads zeroed once
                xbs = [
                    xpool.tile([128, BAND_IN, WP], mm_dt, name=f"xb{b}")
                    for b in range(BANDS)
                ]
                for b in range(BANDS):
                    nc.vector.memzero(xbs[b][:, :, :])

                def load_fix_strips():
                    # xs[32j+c, rho] = X[g, c, rho+j-2, 126]
                    for g in range(IMGS):
                        for j in range(4):
                            lo = max(0, 2 - j)
                            hi = min(H - 1, (H + 1) - j)  # rho max = 129-j clamped
                            nc.sync.dma_start(
                                xs[g][32 * j + 0 : 32 * j + 32, lo : hi + 1, 0],
                                X[g, :, lo + j - 2 : hi + j - 2 + 1, 126],
                            )
                        # xs4[c, rho] = X[g, c, rho+2, 126], rho 0..125
                        nc.sync.dma_start(
                            xs4[g][:, 0 : H - 2, 0], X[g, :, 2:H, 126]
                        )

                def compute_fix():
                    # fix[o, rho] = sum_dy K[o,:,dy,1] * Xp[:, rho+dy, 128]
                    for g in range(IMGS):
                        fp = ppool.tile([O, H, 1], F32, name=f"fix{g}", tag="ps")
                        nc.tensor.matmul(
                            fp[:, :, 0],
                            wtf[:, 0, :],
                            xs[g][:, :, 0],
                            start=True, stop=False,
                            tile_position=(0, 0),
                        )
                        nc.tensor.matmul(
                            fp[:, :, 0],
                            wtf4[:, 0, :],
                            xs4[g][:, :, 0],
                            start=False, stop=True,
                            tile_position=(0, 0),
                        )
                        nc.vector.tensor_copy(fixsb[g][:, :, 0], fp[:, :, 0])

                def band_P(b, xb):
                    y0 = b * BAND_OUT
                    obs = [
                        opool.tile([128, 16, W], out_dt, name=f"ob{b}_{g}", tag="ob")
                        for g in range(IMGS)
                    ]
                    for t in range(BAND_OUT // RT):  # 8 rounds of 4 rows
                        ybase = RT * t
                        ps_g = [
                            ppool.tile([128, RT, W], F32, name=f"ps{b}_{t}_{g}", tag="ps")
                            for g in range(IMGS)
                        ]
                        nslots = 2 * KH + KH  # 10 pairs + 5 solos
                        si = 0
                        for dy in range(KH):
                            for e in range(2):
                                first = si == 0
                                last = si == nslots - 1
                                for g in range(IMGS):
                                    nc.tensor.matmul(
                                        ps_g[g][:, :, :],
                                        wp[32 * g : 32 * g + 32, e, dy, :],
                                        xb[
                                            32 * g : 32 * g + 32,
                                            ybase + dy : ybase + dy + RT,
                                            2 * e : 2 * e + W,
                                        ],
                                        start=first, stop=last,
                                        tile_position=(32 * g, 0),
                                    )
                                si += 1
                        for dy in range(KH):
                            first = si == 0
                            last = si == nslots - 1
                            for g in range(IMGS):
                                nc.tensor.matmul(
                                    ps_g[g][0:O, :, :],
                                    ws[32 * g : 32 * g + 32, dy, :],
                                    xb[
                                        32 * g : 32 * g + 32,
                                        ybase + dy : ybase + dy + RT,
                                        4 : 4 + W,
                                    ],
                                    start=first, stop=last,
                                    tile_position=(32 * g, 0),
                                )
                            si += 1
                        # eviction: TensorTensor may read only one PSUM
                        # operand, so: ACT copies shifted grp1 PSUM->ob,
                        # DVE adds grp0 PSUM onto ob in place.
                        h = t // 4
                        j0 = RT * (t % 4)
                        for g in range(IMGS):
                            dst = obs[g][O * h : O * h + O, j0 : j0 + RT, 0 : W - 1]
                            nc.scalar.activation(
                                dst,
                                ps_g[g][O:128, :, 1:W],
                                mybir.ActivationFunctionType.Copy,
                            )
                            nc.vector.tensor_add(
                                dst, dst, ps_g[g][0:O, :, 0 : W - 1]
                            )
                            nc.vector.tensor_add(
                                obs[g][O * h : O * h + O, j0 : j0 + RT, W - 1 : W],
                                ps_g[g][0:O, :, W - 1 : W],
                                fixsb[g][:, y0 + ybase : y0 + ybase + RT, :],
                            )
                    for g in range(IMGS):
                        dst = out[g, :, y0 : y0 + BAND_OUT, :].rearrange(
                            "o (h j) w -> h o j w", h=2
                        )
                        nc.sync.dma_start(dst, obs[g])

                def body():
                    load_fix_strips()
                    compute_fix()
                    for b in range(BANDS):
                        xb = load_band(b)
                        band_P(b, xb)

            else:
                # ---- scheme A fallback (v3 quadrant baseline) ----
                wt = wpool.tile([128, TAPS, O], mm_dt)
                ksrc = K.rearrange("o c h w -> c (h w) o")
                for g in range(IMGS):
                    nc.sync.dma_start(wt[32 * g : 32 * g + 32, :, :], ksrc)

                def evict(ps_g, y0):
                    for g in range(IMGS):
                        ob = opool.tile([128, RT, W], F32, name="ob", tag="ob")
                        if g % 2 == 0:
                            nc.vector.tensor_copy(ob[:, :, :], ps_g[g][:, :, :])
                        else:
                            nc.scalar.activation(
                                ob[:, :, :], ps_g[g][:, :, :],
                                mybir.ActivationFunctionType.Copy,
                            )
                        # one full-128-partition DMA per tile (vs 2 half-
                        # partition DMAs): partition 64h+o <-> row y0+4h+j
                        dst = out[g, :, y0 : y0 + 2 * RT, :].rearrange(
                            "o (h j) w -> h o j w", h=2
                        )
                        nc.sync.dma_start(dst, ob)

                if SCHEME == "Q64PROBE":
                    # timing-only: 4 quadrant MMs per tap with 64-deep
                    # contraction (sums 2 images' channels -> WRONG output).
                    # Tests whether (64,64)-tile MMs stream 4-way.
                    def body():
                        for b in range(BANDS):
                            xb = load_band(b)
                            for t in range(BAND_OUT // (2 * RT)):
                                ps_g = [
                                    ppool.tile([128, RT, W], F32, name=f"q_{b}_{t}_{i}", tag="ps")
                                    for i in range(2)
                                ]
                                top = 2 * RT * t
                                bot = top + RT
                                for ti, (dy, dx) in enumerate(taps):
                                    first = ti == 0
                                    last = ti == TAPS - 1
                                    for half in range(2):
                                        lhsT = wt[64 * half : 64 * half + 64, dy * KW + dx, :]
                                        rhs_t = xb[
                                            64 * half : 64 * half + 64,
                                            top + dy : top + dy + RT,
                                            dx : dx + W,
                                        ]
                                        rhs_b = xb[
                                            64 * half : 64 * half + 64,
                                            bot + dy : bot + dy + RT,
                                            dx : dx + W,
                                        ]
                                        # top blocks on the diagonal quadrants
                                        nc.tensor.matmul(
                                            ps_g[0][64 * half : 64 * half + 64, :, :],
                                            lhsT, rhs_t,
                                            start=first, stop=last,
                                            tile_position=(64 * half, 64 * half),
                                        )
                                        # bottom blocks on the anti-diagonal
                                        nc.tensor.matmul(
                                            ps_g[1][
                                                64 * (1 - half) : 64 * (1 - half) + 64, :, :
                                            ],
                                            lhsT, rhs_b,
                                            start=first, stop=last,
                                            tile_position=(64 * half, 64 * (1 - half)),
                                        )
                                evict(ps_g + ps_g, b * BAND_OUT + 2 * RT * t)
                else:
                    def body():
                        for b in range(BANDS):
                            xb = load_band(b)
                            for t in range(BAND_OUT // (2 * RT)):
                                ps_g = [
                                    ppool.tile([128, RT, W], F32, name=f"ps_{b}_{t}_{g}", tag="ps")
                                    for g in range(IMGS)
                                ]
                                top = 2 * RT * t
                                bot = top + RT
                                for ti, (dy, dx) in enumerate(taps):
                                    first = ti == 0
                                    last = ti == TAPS - 1
                                    for g in range(IMGS):
                                        lhsT = wt[32 * g : 32 * g + 32, dy * KW + dx, :]
                                        rhs_t = xb[
                                            32 * g : 32 * g + 32,
                                            top + dy : top + dy + RT,
                                            dx : dx + W,
                                        ]
                                        rhs_b = xb[
                                            32 * g : 32 * g + 32,
                                            bot + dy : bot + dy + RT,
                                            dx : dx + W,
                                        ]
                                        nc.tensor.matmul(
                                            ps_g[g][0:O, :, :], lhsT, rhs_t,
                                            start=first, stop=last,
                                            tile_position=(32 * g, 0),
                                        )
                                        nc.tensor.matmul(
                                            ps_g[g][O:128, :, :], lhsT, rhs_b,
                                            start=first, stop=last,
                                            tile_position=(32 * g, O),
                                        )
                                evict(ps_g, b * BAND_OUT + 2 * RT * t)

            if reps > 1:
                with tc.For_i(0, reps, 1):
                    body()
            else:
                body()
    nc.compile()
    return nc


_CACHE = {}


def _get_nc(reps=1):
    if reps not in _CACHE:
        _CACHE[reps] = _build_nc(reps)
    return _CACHE[reps]


def make_in_maps(X, K):
    dt = ml_dtypes.bfloat16
    X = np.ascontiguousarray(np.asarray(X), dtype=np.float32)
    K = np.ascontiguousarray(np.asarray(K), dtype=np.float32)
    per = X.shape[0] // N_CORES
    Z = np.zeros((128, BAND_IN, 2 * PAD), dtype=dt)
    ZR = np.zeros((128, PAD, WP), dtype=dt)
    Kc = K.astype(dt)
    return [
        {
            "X": np.ascontiguousarray(X[per * i : per * (i + 1)]).astype(dt),
            "K": Kc,
            "Z": Z,
            "ZR": ZR,
        }
        for i in range(N_CORES)
    ]


def kernel(X, K):
    nc = _get_nc()
    in_maps = make_in_maps(X, K)
    res = run_bass_kernel_spmd(nc, in_maps, list(range(N_CORES))).results
    return np.concatenate(
        [res[i]["out"].astype(np.float32) for i in range(N_CORES)], axis=0
    )
